# revision 1
# baseline (speedup 1.0000x reference)
"""Trainium2 Bass kernel for nn_AttentionBlock (adaLN-modulated GroupNorm attention).

Sharding: data-parallel over batch B=8 -> one batch per NeuronCore (8 cores).
Each core runs the full block for its batch:
  groupnorm(32 groups) -> adaLN modulate -> qkv matmul -> 8-head attention
  (softmax over keys) -> proj matmul -> gated residual.

Layouts (per core):
  x (fp32), xm (bf16):  [C=512, T=1024] as 4 tiles [128, 1024], channels on
                        partitions.
  qkv out (bf16): 12 tiles [128, 1024], channel order PERMUTED to type-major
              [q0..q7 | k0..k7 | v0..v7] (64 rows per head-type block) so that
              q_h and k_h always live at the same partition offset (0 or 64).
  scoresT:    [s, t] per head ([128 s, 1024 t] PSUM fp32), one batched exp on
              ScalarE fuses the PSUM->SBUF eviction (out bf16).
  PV:         U[65, t] = [vT | ones].T @ expT  -> row 64 is the softmax denom.
  normalize:  1/denom via reciprocal_approx_fast on partition 0, broadcast
              across partitions with gpsimd.partition_broadcast.

Matmuls run in bf16 (fp32 PSUM accumulation); groupnorm statistics stay fp32.
"""

import numpy as np

import concourse.bass as bass
import concourse.tile as tile
from concourse import bacc, mybir
from concourse.bass_utils import run_bass_kernel_spmd

AF = mybir.ActivationFunctionType
f32 = mybir.dt.float32
bf16 = mybir.dt.bfloat16

B, C, HH, WW, E = 8, 512, 32, 32, 512
HEADS, G = 8, 32
T = HH * WW          # 1024
CH = C // HEADS      # 64
NC_ = C // 128       # 4 channel chunks
NO = 3 * C // 128    # 12 qkv output chunks
NT = T // 512        # 2 t-chunks of 512
NS = T // 128        # 8 s-chunks of 128
EPS = 1e-5


def _perm():
    """new[512*ty + 64*h + r] = orig[192*h + 64*ty + r] (head-major -> type-major)."""
    p = np.empty(3 * C, np.int64)
    for h in range(HEADS):
        for ty in range(3):
            p[512 * ty + 64 * h : 512 * ty + 64 * h + 64] = (
                192 * h + 64 * ty + np.arange(64)
            )
    return p


def _build_program():
    nc = bacc.Bacc("TRN2", target_bir_lowering=False, debug=False, num_devices=8)

    # ---- DRAM parameters (per-core shards; weights replicated, bf16) ----
    x_d = nc.declare_dram_parameter("x", [C, T], f32, isOutput=False)
    emb_d = nc.declare_dram_parameter("emb", [E], f32, isOutput=False)
    qw_d = nc.declare_dram_parameter("qkv_wT", [C, 3 * C], bf16, isOutput=False)
    qb_d = nc.declare_dram_parameter("qkv_b", [3 * C], f32, isOutput=False)
    aw_d = nc.declare_dram_parameter("ada_wT", [E, 3 * C], bf16, isOutput=False)
    ab_d = nc.declare_dram_parameter("ada_b", [3 * C], f32, isOutput=False)
    pw_d = nc.declare_dram_parameter("proj_wT", [C, C], bf16, isOutput=False)
    pb_d = nc.declare_dram_parameter("proj_b", [C], f32, isOutput=False)
    gind_d = nc.declare_dram_parameter("gind", [128, 8], f32, isOutput=False)
    gindT_d = nc.declare_dram_parameter("gindT", [8, 128], f32, isOutput=False)
    ident_d = nc.declare_dram_parameter("ident", [128, 128], bf16, isOutput=False)
    ones_d = nc.declare_dram_parameter("ones", [128, 64], f32, isOutput=False)
    out_d = nc.declare_dram_parameter("out", [C, T], f32, isOutput=True)

    from contextlib import ExitStack

    with tile.TileContext(nc) as tc, ExitStack() as ctx:
        ctx.enter_context(
            nc.allow_low_precision(reason="bf16 matmul inputs; fp32 accumulate")
        )
        P = ctx.enter_context(tc.tile_pool(name="persist", bufs=1))
        # one shared PSUM tag: 2 rotating [128,1024] fp32 slots (4 banks)
        PSM = ctx.enter_context(tc.tile_pool(name="psm", bufs=2, space="PSUM"))
        PSU = ctx.enter_context(tc.tile_pool(name="psu", bufs=4, space="PSUM"))

        # ---- persistent SBUF tiles + input DMAs ----
        gind_sb = P.tile([128, 8], f32, tag="gind")
        gindT_sb = P.tile([8, 128], f32, tag="gindT")
        ident_sb = P.tile([128, 128], bf16, tag="ident")
        ones_sb = P.tile([128, 64], f32, tag="ones")
        emb_sb = P.tile([128, 4], f32, tag="emb")
        silu_sb = P.tile([128, 4], bf16, tag="silu")
        qb_sb = P.tile([128, 12], f32, tag="qb")
        ab_sb = P.tile([128, 12], f32, tag="ab")
        pb_sb = P.tile([128, 4], f32, tag="pb")
        mod_sb = P.tile([128, 12], f32, tag="mod")

        nc.sync.dma_start(out=gind_sb, in_=gind_d.ap())
        nc.sync.dma_start(out=gindT_sb, in_=gindT_d.ap())
        nc.sync.dma_start(out=ident_sb, in_=ident_d.ap())
        nc.sync.dma_start(out=ones_sb, in_=ones_d.ap())
        nc.sync.dma_start(out=emb_sb, in_=emb_d.ap().rearrange("(f p) -> p f", p=128))
        nc.sync.dma_start(out=qb_sb, in_=qb_d.ap().rearrange("(f p) -> p f", p=128))
        nc.sync.dma_start(out=ab_sb, in_=ab_d.ap().rearrange("(f p) -> p f", p=128))
        nc.sync.dma_start(out=pb_sb, in_=pb_d.ap().rearrange("(f p) -> p f", p=128))

        xf = []
        for i in range(NC_):
            t_ = P.tile([128, T], f32, tag=f"xf{i}")
            nc.sync.dma_start(out=t_, in_=x_d.ap()[128 * i : 128 * (i + 1), :])
            xf.append(t_)
        awp_cm = tc.tile_pool(name="awp", bufs=1)
        AWP = awp_cm.__enter__()
        aw = []
        for j in range(4):
            t_ = AWP.tile([128, 3 * C], bf16, tag=f"aw{j}", name=f"aw{j}")
            nc.sync.dma_start(out=t_, in_=aw_d.ap()[128 * j : 128 * (j + 1), :])
            aw.append(t_)
        qw = []
        for j in range(4):
            t_ = P.tile([128, 3 * C], bf16, tag=f"qw{j}")
            nc.sync.dma_start(out=t_, in_=qw_d.ap()[128 * j : 128 * (j + 1), :])
            qw.append(t_)
        pw = []
        for j in range(4):
            t_ = P.tile([128, C], bf16, tag=f"pw{j}")
            nc.sync.dma_start(out=t_, in_=pw_d.ap()[128 * j : 128 * (j + 1), :])
            pw.append(t_)

        # ---- phase 1: adaLN modulation (PE) + groupnorm stats (DVE) ----
        sg_sb = P.tile([128, 4], f32, tag="sg")
        nc.scalar.activation(sg_sb, emb_sb, AF.Sigmoid)
        nc.vector.tensor_mul(silu_sb, emb_sb, sg_sb)
        # mod^T = silu^T @ ada_wT as [1, 1536], then DRAM-bounce to [128, 12]
        mrow = P.tile([1, 3 * C], f32, tag="mrow")
        for oc in range(3):
            mps = PSM.tile([1, 512], f32, tag="sc", name=f"mps{oc}")
            for j in range(4):
                nc.tensor.matmul(
                    mps,
                    silu_sb[:, j : j + 1],
                    aw[j][:, 512 * oc : 512 * (oc + 1)],
                    start=(j == 0),
                    stop=(j == 3),
                )
            nc.vector.tensor_copy(mrow[:, 512 * oc : 512 * (oc + 1)], mps)
        awp_cm.__exit__(None, None, None)
        EXPP = ctx.enter_context(tc.tile_pool(name="expp", bufs=6))
        ANP = ctx.enter_context(tc.tile_pool(name="anp", bufs=4))
        modp_sb = P.tile([128, 12], f32, tag="modp")
        # partition-scatter via DRAM bounce (SBUF partition dim is physical)
        DP = ctx.enter_context(tc.tile_pool(name="dramp", bufs=1, space="DRAM"))
        mod_scr = DP.tile([1, 3 * C], f32, tag="modscr")
        nc.sync.dma_start(out=mod_scr, in_=mrow)
        nc.sync.dma_start(
            out=modp_sb, in_=mod_scr[0, :].rearrange("(f p) -> p f", p=128)
        )
        nc.vector.tensor_add(mod_sb, modp_sb, ab_sb)

        mv = []
        for i in range(NC_):
            st6 = P.tile([128, 2, 6], f32, tag=f"st6{i}")
            xv = xf[i][:].rearrange("p (s f) -> p s f", f=512)
            for si in range(2):
                nc.vector.bn_stats(st6[:, si, :], xv[:, si, :])
            mv_i = P.tile([128, 2], f32, tag=f"mv{i}")
            nc.vector.bn_aggr(mv_i, st6)
            # E2 = var + mu^2 into col 1
            tm = P.tile([128, 1], f32, tag=f"tmu{i}")
            nc.vector.tensor_mul(tm, mv_i[:, 0:1], mv_i[:, 0:1])
            nc.vector.tensor_add(mv_i[:, 1:2], mv_i[:, 1:2], tm)
            mv.append(mv_i)

        stats8_ps = PSM.tile([8, 8], f32, tag="sc", name="stats8")
        for i in range(NC_):
            nc.tensor.matmul(
                stats8_ps[:, 2 * i : 2 * i + 2], gind_sb, mv[i], start=True, stop=True
            )
        s8 = P.tile([8, 8], f32, tag="s8")
        nc.vector.tensor_copy(s8, stats8_ps)
        musq8 = P.tile([8, 4], f32, tag="musq8")
        var8 = P.tile([8, 4], f32, tag="var8")
        sd8 = P.tile([8, 4], f32, tag="sd8")
        rstd8 = P.tile([8, 4], f32, tag="rstd8")
        for i in range(NC_):
            nc.vector.tensor_mul(
                musq8[:, i : i + 1], s8[:, 2 * i : 2 * i + 1], s8[:, 2 * i : 2 * i + 1]
            )
            nc.vector.tensor_sub(
                var8[:, i : i + 1], s8[:, 2 * i + 1 : 2 * i + 2], musq8[:, i : i + 1]
            )
        eps8 = P.tile([8, 1], f32, tag="eps8")
        nc.vector.memset(eps8, EPS)
        nc.scalar.activation(sd8, var8, AF.Sqrt, bias=eps8)
        nc.vector.reciprocal(rstd8, sd8)

        xm = []
        for i in range(NC_):
            statbc = PSM.tile([128, 2], f32, tag="sc", name=f"statbc{i}")
            nc.tensor.matmul(
                statbc[:, 0:1], gindT_sb, s8[:, 2 * i : 2 * i + 1], start=True, stop=True
            )
            nc.tensor.matmul(
                statbc[:, 1:2], gindT_sb, rstd8[:, i : i + 1], start=True, stop=True
            )
            s1p = P.tile([128, 1], f32, tag=f"s1p{i}")
            A_i = P.tile([128, 1], f32, tag=f"A{i}")
            B_i = P.tile([128, 1], f32, tag=f"B{i}")
            tm2 = P.tile([128, 1], f32, tag=f"tm2{i}")
            nc.vector.tensor_scalar_add(s1p, mod_sb[:, 4 + i : 5 + i], 1.0)
            nc.vector.tensor_mul(A_i, statbc[:, 1:2], s1p)
            nc.vector.tensor_mul(tm2, statbc[:, 0:1], A_i)
            nc.vector.tensor_sub(B_i, mod_sb[:, i : i + 1], tm2)
            xm_i = P.tile([128, T], bf16, tag=f"xm{i}")
            nc.scalar.activation(xm_i, xf[i], AF.Identity, bias=B_i, scale=A_i)
            xm.append(xm_i)

        # ---- phase 2: qkv matmul [1536, 1024] (channel order = type-major) ----
        qkv = [P.tile([128, T], bf16, tag=f"qkv{m}", name=f"qkv{m}") for m in range(NO)]
        # chunk order: all three chunks of head pair 0 first, then pair 1, ...
        m_order = [p + 4 * ty for p in range(4) for ty in range(3)]
        for m in m_order:
            ps = PSM.tile([128, T], f32, tag="sc", name=f"qkvps{m}")
            for t in range(NT):
                for j in range(4):
                    nc.tensor.matmul(
                        ps[:, 512 * t : 512 * (t + 1)],
                        qw[j][:, 128 * m : 128 * (m + 1)],
                        xm[j][:, 512 * t : 512 * (t + 1)],
                        start=(j == 0),
                        stop=(j == 3),
                    )
            nc.vector.tensor_scalar_add(qkv[m][:], ps, qb_sb[:, m : m + 1])

        # ---- phase 3+4: attention, head pairs interleaved ----
        # Heads 2j / 2j+1 live at partition offsets 0 / 64 of the same qkv
        # tiles; interleaving their K=64 matmuls puts them in different PE
        # row-groups so they can execute concurrently.
        a_sb = [
            P.tile([128, T], bf16, tag=f"asb{j}", name=f"asb{j}") for j in range(NC_)
        ]
        vT = [
            P.tile([128, 8, 65], bf16, tag=f"vt{h}", name=f"vt{h}")
            for h in range(HEADS)
        ]
        for hp in range(4):
            heads = (2 * hp, 2 * hp + 1)
            for h in heads:
                nc.vector.tensor_copy(
                    vT[h][:, :, 64:65],
                    ones_sb[:, 0:8].rearrange("p (a o) -> p a o", o=1),
                )
            for s in range(NS):
                for h in heads:
                    off = 64 * (h % 2)
                    v_ap = qkv[8 + h // 2][off : off + 64, :]
                    vtr = PSM.tile([128, 64], bf16, tag="sc", name=f"vtr{hp}_{s}_{h}")
                    nc.tensor.transpose(
                        vtr,
                        v_ap[:, 128 * s : 128 * (s + 1)],
                        ident_sb[off : off + 64, off : off + 64],
                        tile_position=(off, 0),
                    )
                    nc.vector.tensor_copy(vT[h][:, s, 0:64], vtr)
            U = {}
            for h in heads:
                for t in range(NT):
                    U[(h, t)] = PSU.tile([65, 512], f32, tag="u", name=f"u{h}_{t}")
            ex_tiles = {}
            for s in range(NS):
                for h in heads:
                    off = 64 * (h % 2)
                    q_ap = qkv[h // 2][off : off + 64, :]
                    k_ap = qkv[4 + h // 2][off : off + 64, :]
                    sc = PSM.tile([128, T], f32, tag="sc", name=f"sc{hp}_{s}_{h}")
                    for t in range(NT):
                        nc.tensor.matmul(
                            sc[:, 512 * t : 512 * (t + 1)],
                            k_ap[:, 128 * s : 128 * (s + 1)],
                            q_ap[:, 512 * t : 512 * (t + 1)],
                            start=True,
                            stop=True,
                            tile_position=(off, 0),
                        )
                    ex = EXPP.tile([128, T], bf16, tag="ex")
                    nc.scalar.activation(ex, sc, AF.Exp, scale=0.125)
                    ex_tiles[(h, s)] = ex
                if s >= 1:
                    for h in heads:
                        ex = ex_tiles.pop((h, s - 1))
                        for t in range(NT):
                            nc.tensor.matmul(
                                U[(h, t)],
                                vT[h][:, s - 1, :],
                                ex[:, 512 * t : 512 * (t + 1)],
                                start=(s - 1 == 0),
                                stop=False,
                            )
            for h in heads:
                ex = ex_tiles.pop((h, NS - 1))
                for t in range(NT):
                    nc.tensor.matmul(
                        U[(h, t)],
                        vT[h][:, NS - 1, :],
                        ex[:, 512 * t : 512 * (t + 1)],
                        start=False,
                        stop=True,
                    )
            # normalize: a = U[0:64] / denom (denom = row 64); the reciprocal
            # runs on partition 0 (partition_broadcast sources partition 0)
            for h in heads:
                off = 64 * (h % 2)
                for t in range(NT):
                    rc = ANP.tile([65, 512], f32, tag="rc", bufs=2)
                    nc.vector.tensor_copy(rc[64:65, :], U[(h, t)][64:65, :])
                    rc0 = ANP.tile([1, 512], f32, tag="rc0", bufs=2)
                    nc.sync.dma_start(out=rc0, in_=rc[64:65, :])
                    nc.vector.reciprocal_approx_fast(out=rc0[:], in_=rc0[:])
                    rbs = ANP.tile([64, 512], f32, tag="rbs")
                    nc.gpsimd.partition_broadcast(rbs[:], rc0[:])
                    abf = ANP.tile([64, 512], bf16, tag="abf")
                    nc.vector.tensor_mul(abf, U[(h, t)][0:64, :], rbs)
                    nc.sync.dma_start(
                        out=a_sb[h // 2][off : off + 64, 512 * t : 512 * (t + 1)],
                        in_=abf,
                    )

        # ---- phase 5: proj + gated residual ----
        pbg = []
        for i_ in range(NC_):
            t_ = P.tile([128, 1], f32, tag=f"pbg{i_}")
            nc.vector.tensor_mul(t_, pb_sb[:, i_ : i_ + 1], mod_sb[:, 8 + i_ : 9 + i_])
            pbg.append(t_)
        for m in range(NC_):
            ps = PSM.tile([128, T], f32, tag="sc", name=f"projps{m}")
            for t in range(NT):
                for j in range(4):
                    nc.tensor.matmul(
                        ps[:, 512 * t : 512 * (t + 1)],
                        pw[j][:, 128 * m : 128 * (m + 1)],
                        a_sb[j][:, 512 * t : 512 * (t + 1)],
                        start=(j == 0),
                        stop=(j == 3),
                    )
            tg = ANP.tile([128, T], f32, tag="tg", bufs=2)
            nc.scalar.activation(
                tg, ps, AF.Identity, bias=pbg[m], scale=mod_sb[:, 8 + m : 9 + m]
            )
            # residual in-place into xf (xf never feeds a matmul)
            nc.vector.tensor_add(xf[m][:], xf[m][:], tg)
            nc.sync.dma_start(out=out_d.ap()[128 * m : 128 * (m + 1), :], in_=xf[m])

    nc.compile()
    return nc


_PROGRAM = None
LAST_RESULTS = None


def _get_program():
    global _PROGRAM
    if _PROGRAM is None:
        _PROGRAM = _build_program()
    return _PROGRAM


def kernel(x, emb, qkv_w, qkv_b, ada_w, ada_b, proj_w, proj_b, _trace=False):
    global LAST_RESULTS
    import ml_dtypes

    nc = _get_program()

    x = np.asarray(x, np.float32)
    emb = np.asarray(emb, np.float32)
    perm = _perm()
    bf = ml_dtypes.bfloat16
    qkv_wT = np.ascontiguousarray(np.asarray(qkv_w, np.float32)[perm, :].T.astype(bf))
    qkv_b_p = np.ascontiguousarray(np.asarray(qkv_b, np.float32)[perm])
    ada_wT = np.ascontiguousarray(np.asarray(ada_w, np.float32).T.astype(bf))
    ada_b = np.ascontiguousarray(np.asarray(ada_b, np.float32))
    proj_wT = np.ascontiguousarray(np.asarray(proj_w, np.float32).T.astype(bf))
    proj_b = np.ascontiguousarray(np.asarray(proj_b, np.float32))

    gind = np.repeat(np.eye(8, dtype=np.float32), 16, axis=0) / 16.0  # [128, 8]
    gindT = np.ascontiguousarray(np.repeat(np.eye(8, dtype=np.float32), 16, axis=0).T)
    ident = np.eye(128, dtype=bf)
    ones = np.ones((128, 64), dtype=np.float32)

    in_maps = []
    for b in range(B):
        in_maps.append(
            {
                "x": np.ascontiguousarray(x[b].reshape(C, T)),
                "emb": np.ascontiguousarray(emb[b]),
                "qkv_wT": qkv_wT,
                "qkv_b": qkv_b_p,
                "ada_wT": ada_wT,
                "ada_b": ada_b,
                "proj_wT": proj_wT,
                "proj_b": proj_b,
                "gind": gind,
                "gindT": gindT,
                "ident": ident,
                "ones": ones,
            }
        )

    res = run_bass_kernel_spmd(nc, in_maps, list(range(8)), trace=_trace)
    LAST_RESULTS = res
    out = np.stack([res.results[b]["out"] for b in range(B)], axis=0)
    return np.ascontiguousarray(out.reshape(B, C, HH, WW).astype(np.float32))



# revision 18
# speedup vs baseline: 1.1585x; 1.1585x over previous
"""Trainium2 Bass kernel for nn_AttentionBlock (adaLN-modulated GroupNorm attention).

Sharding: data-parallel over batch B=8 -> one batch per NeuronCore (8 cores).
Each core runs the full block for its batch:
  groupnorm(32 groups) -> adaLN modulate -> qkv matmul -> 8-head attention
  (softmax over keys) -> proj matmul -> gated residual.

v2 design notes (vs the first working version):
  * The TRN2 PE clock ramps 0.65 -> 1.2 -> 2.4 GHz only while continuously
    busy; any idle gap drops it back.  The schedule therefore keeps the PE
    saturated: cheap "filler" matmuls (zero lhsT column into an unused PSUM
    row) bridge every unavoidable dependency gap.
  * ScalarE (ACT) runs softmax exp only -- everything else that the baseline
    ran there (silu, sqrt, modulate, output scale) moved to DVE/Pool or the
    exp/ln activation table, so there is a single act-table load.
  * V^T comes from dma_start_transpose on the DMA engines, not PE transposes.
  * softmax denominators ride the PV matmul as a trailing ones-column of
    vT (row 64 of U); the reciprocal is taken in place on partition 64 and
    broadcast across partitions 0..63 with a K=1 outer-product matmul.
  * Layouts per core:
      x (fp32), xm (bf16): [C=512, T=1024] as 4 tiles [128, 1024]
      qkv (bf16): 12 tiles [128, 1024], channel order PERMUTED type-major
                  [q0..q7 | k0..k7 | v0..v7], head pair j at rows 0/64.
      scoresT [s, t] per (head, s-chunk): [128, 1024] PSUM -> one exp each.
      U [97, 1024] PSUM per head: rows 0..63 PV, row 64 denom, row 96 filler.
"""

import numpy as np

import concourse.bass as bass
import concourse.tile as tile
from concourse import bacc, mybir
from concourse.bass_utils import run_bass_kernel_spmd

AF = mybir.ActivationFunctionType
ALU = mybir.AluOpType
f32 = mybir.dt.float32
bf16 = mybir.dt.bfloat16

B, C, HH, WW, E = 8, 512, 32, 32, 512
HEADS, G = 8, 32
T = HH * WW          # 1024
CH = C // HEADS      # 64
NC_ = C // 128       # 4 channel chunks
NO = 3 * C // 128    # 12 qkv output chunks
NT = T // 512        # 2 t-chunks of 512
NS = T // 128        # 8 s-chunks of 128
EPS = 1e-5

# filler tuning: matmuls that only exist to keep the PE clock ramped
FILL_WARM = 6       # phase A (between prep matmuls and qkv)
FILL_C = 1           # per attention s-iteration
FILL_BOUND = 5       # at each head-pair boundary
FILL_TAIL = 6        # before proj


def _perm():
    """new[512*ty + 64*h + r] = orig[192*h + 64*ty + r] (head-major -> type-major)."""
    p = np.empty(3 * C, np.int64)
    for h in range(HEADS):
        for ty in range(3):
            p[512 * ty + 64 * h : 512 * ty + 64 * h + 64] = (
                192 * h + 64 * ty + np.arange(64)
            )
    return p


def _build_program():
    nc = bacc.Bacc("TRN2", target_bir_lowering=False, debug=False, num_devices=8)

    # ---- DRAM parameters (per-core shards; weights replicated, bf16) ----
    # smalls [128, 40] packs emb(0:4) qb(4:16) ab(16:28) pb(28:32) gind(32:40)
    x_d = nc.declare_dram_parameter("x", [C, T], f32, isOutput=False)
    smalls_d = nc.declare_dram_parameter("smalls", [128, 40], f32, isOutput=False)
    qw_d = nc.declare_dram_parameter("qkv_wT", [C, 3 * C], bf16, isOutput=False)
    aw_d = nc.declare_dram_parameter("ada_wT", [E, 3 * C], bf16, isOutput=False)
    pw_d = nc.declare_dram_parameter("proj_wT", [C, C], bf16, isOutput=False)
    gindT_d = nc.declare_dram_parameter("gindT", [8, 128], f32, isOutput=False)
    out_d = nc.declare_dram_parameter("out", [C, T], f32, isOutput=True)

    from contextlib import ExitStack

    with tile.TileContext(nc) as tc, ExitStack() as ctx:
        ctx.enter_context(
            nc.allow_low_precision(reason="bf16 matmul inputs; fp32 accumulate")
        )
        P = ctx.enter_context(tc.tile_pool(name="persist", bufs=1))
        # PSUM: 'sc' 2x[128,1024] (4 banks) + 'u' 2x[97,1024] (4 banks) = all 8
        SC = ctx.enter_context(tc.tile_pool(name="scp", bufs=2, space="PSUM"))
        UU = ctx.enter_context(tc.tile_pool(name="uup", bufs=2, space="PSUM"))
        EXPP = ctx.enter_context(tc.tile_pool(name="expp", bufs=6))
        RECP = ctx.enter_context(tc.tile_pool(name="recp", bufs=2))
        ATMP = ctx.enter_context(tc.tile_pool(name="atmp", bufs=2))
        OUTP = ctx.enter_context(tc.tile_pool(name="outp", bufs=2))
        DP = ctx.enter_context(tc.tile_pool(name="dramp", bufs=1, space="DRAM"))

        # ---- persistent SBUF tiles + input DMAs ----
        # DMA dispatch costs ~0.7us on the issuing engine's queue, so the
        # count matters and the issue load is split SP (nc.sync) / ACT
        # (nc.scalar): ACT is idle through phases A/B.
        smalls = P.tile([128, 40], f32, tag="smalls")
        gindT_sb = P.tile([8, 128], f32, tag="gindT")
        nc.sync.dma_start(out=smalls, in_=smalls_d.ap())
        nc.sync.dma_start(out=gindT_sb, in_=gindT_d.ap())
        emb_sb = smalls[:, 0:4]
        qb_sb = smalls[:, 4:16]
        ab_sb = smalls[:, 16:28]
        pb_sb = smalls[:, 28:32]
        gind_sb = smalls[:, 32:40]

        xf = []
        for i in range(NC_):
            t_ = P.tile([128, T], f32, tag=f"xf{i}")
            nc.sync.dma_start(out=t_, in_=x_d.ap()[128 * i : 128 * (i + 1), :])
            xf.append(t_)
        qw = [P.tile([128, 3 * C], bf16, tag=f"qw{j}", name=f"qw{j}") for j in range(4)]
        for j in range(4):
            nc.sync.dma_start(out=qw[j], in_=qw_d.ap()[128 * j : 128 * (j + 1), :])
        # force the ln+exp act table once, before anything queues on ACT
        dummy1 = P.tile([1, 1], f32, tag="dummy1")
        nc.vector.memset(dummy1, 1.0)
        nc.scalar.activation(dummy1, dummy1, AF.Ln)
        aw = [P.tile([128, 3 * C], bf16, tag=f"aw{j}", name=f"aw{j}") for j in range(4)]
        nc.scalar.dma_start(out=aw[0], in_=aw_d.ap()[0:128, :])
        eneg = P.tile([128, 4], f32, tag="eneg")
        nc.scalar.activation(eneg, emb_sb, AF.Exp, scale=-1.0)
        for j in range(1, 4):
            nc.scalar.dma_start(out=aw[j], in_=aw_d.ap()[128 * j : 128 * (j + 1), :])
        pw = [P.tile([128, C], bf16, tag=f"pw{j}", name=f"pw{j}") for j in range(4)]

        # ---- constants (memset; no DMA needed) ----
        ones_t = P.tile([65, 64], bf16, tag="ones_t")
        nc.vector.memset(ones_t, 1.0)
        zc = P.tile([128, 1], bf16, tag="zc")
        nc.vector.memset(zc, 0.0)
        fill_src = P.tile([128, 512], bf16, tag="fillsrc")
        nc.gpsimd.memset(fill_src, 0.0)
        # vT[h][p, s, 0:64] = v_h[c, 128s+p]; col 64 = ones (rides the v80
        # source row 64 through the XBAR transpose); cols 65:80 = pad.
        vT = [P.tile([128, NS, 80], bf16, tag=f"vt{h}", name=f"vt{h}") for h in range(HEADS)]
        v80 = [P.tile([80, T], bf16, tag=f"v80_{h}", name=f"v80_{h}") for h in range(HEADS)]
        for h in range(HEADS):
            nc.gpsimd.memset(v80[h], 1.0)
        vstage = [
            P.tile([128, T], bf16, tag=f"vstage{p}", name=f"vstage{p}")
            for p in range(4)
        ]

        # warm-up / filler machinery ------------------------------------
        warm_u = UU.tile([97, T], f32, tag="u", name="warm_u")
        _filln = [0]

        def fill(n, utile=None):
            dst = warm_u if utile is None else utile
            for _ in range(n):
                _filln[0] += 1
                nc.tensor.matmul(
                    dst[96:97, 0:512],
                    zc,
                    fill_src,
                    start=False,
                    stop=True,
                    tile_position=(0, 96),
                    skip_group_check=True,
                )

        # ramp the PE to max p-state while phase A's latency chain plays out
        fill(10)

        # ---- phase A: silu -> mod row (PE), groupnorm stats (DVE) ----
        # silu(x) = x / (1 + exp(-x)); eneg was issued above, right after the
        # act-table prime.
        sden = P.tile([128, 4], f32, tag="sden")
        nc.vector.tensor_scalar_add(sden, eneg, 1.0)
        srec = P.tile([128, 4], f32, tag="srec")
        nc.vector.reciprocal(srec, sden)
        silu_sb = P.tile([128, 4], bf16, tag="silu")
        nc.vector.tensor_mul(silu_sb, emb_sb, srec)

        # mod^T = silu^T @ ada_wT as [1, 1536] (PE warms up here)
        mrow = P.tile([1, 3 * C], f32, tag="mrow")
        for oc in range(3):
            mps = SC.tile([128, T], f32, tag="sc", name=f"mps{oc}")
            for j in range(4):
                nc.tensor.matmul(
                    mps[0:1, 0:512],
                    silu_sb[:, j : j + 1],
                    aw[j][:, 512 * oc : 512 * (oc + 1)],
                    start=(j == 0),
                    stop=(j == 3),
                )
            nc.vector.tensor_copy(mrow[:, 512 * oc : 512 * (oc + 1)], mps[0:1, 0:512])
            fill(1)

        # partition-scatter via DRAM bounce (SBUF partition dim is physical)
        modp_sb = P.tile([128, 12], f32, tag="modp")
        mod_scr = DP.tile([1, 3 * C], f32, tag="modscr")
        nc.scalar.dma_start(out=mod_scr, in_=mrow)
        nc.scalar.dma_start(
            out=modp_sb, in_=mod_scr[0, :].rearrange("(f p) -> p f", p=128)
        )
        mod_sb = P.tile([128, 12], f32, tag="mod")
        nc.vector.tensor_add(mod_sb, modp_sb, ab_sb)

        # groupnorm stats: bn_stats/aggr per chunk -> mv[:, i, 0]=mu, [.,1]=E2
        mv = P.tile([128, 4, 2], f32, tag="mv")
        st6 = P.tile([128, 2, 6], f32, tag="st6")
        for i in range(NC_):
            xv = xf[i][:].rearrange("p (s f) -> p s f", f=512)
            for si in range(2):
                nc.vector.bn_stats(st6[:, si, :], xv[:, si, :])
            nc.vector.bn_aggr(mv[:, i, :], st6)
            # E2 = var + mu^2 in one fused op: (mu * mu) + var
            nc.vector.scalar_tensor_tensor(
                mv[:, i, 1:2], mv[:, i, 0:1], mv[:, i, 0:1], mv[:, i, 1:2],
                ALU.mult, ALU.add,
            )

        fill(2)
        # group-reduce 16 partitions per group-block: [8, 8] = (mu|E2) x 4
        stats_ps = SC.tile([128, T], f32, tag="sc", name="statsps")
        nc.tensor.matmul(
            stats_ps[0:8, 0:8],
            gind_sb,
            mv[:].rearrange("p a b -> p (a b)"),
            start=True,
            stop=True,
        )
        # rhs8 cols 0..3 = mu per chunk, cols 4..7 = rstd per chunk
        sview = stats_ps[0:8, 0:8].rearrange("p (a b) -> p a b", b=2)
        rhs8 = P.tile([8, 8], f32, tag="rhs8")
        nc.vector.tensor_copy(rhs8[:, 0:4], sview[:, :, 0])
        # var = E2 - mu^2 straight from PSUM
        mu2 = P.tile([8, 4], f32, tag="mu2")
        nc.vector.tensor_mul(mu2, rhs8[:, 0:4], rhs8[:, 0:4])
        var8 = P.tile([8, 4], f32, tag="var8")
        nc.vector.tensor_sub(var8, sview[:, :, 1], mu2)
        # rstd = exp(-0.5 * ln(var + eps)) -- stays on the exp/ln table
        eps8 = P.tile([8, 1], f32, tag="eps8")
        nc.vector.memset(eps8, EPS)
        lnv = P.tile([8, 4], f32, tag="lnv")
        nc.scalar.activation(lnv, var8, AF.Ln, bias=eps8)
        nc.scalar.activation(rhs8[:, 4:8], lnv, AF.Exp, scale=-0.5)
        fill(2)
        statbc_ps = SC.tile([128, T], f32, tag="sc", name="statbc")
        nc.tensor.matmul(statbc_ps[0:128, 0:8], gindT_sb, rhs8, start=True, stop=True)
        # A = rstd*(1+scale); B = shift - mu*A  (per channel)
        s1p = P.tile([128, 4], f32, tag="s1p")
        nc.vector.tensor_scalar_add(s1p, mod_sb[:, 4:8], 1.0)
        A4 = P.tile([128, 4], f32, tag="A4")
        nc.vector.tensor_mul(A4, statbc_ps[0:128, 4:8], s1p)
        muA = P.tile([128, 4], f32, tag="muA")
        nc.vector.tensor_mul(muA, statbc_ps[0:128, 0:4], A4)
        B4 = P.tile([128, 4], f32, tag="B4")
        nc.vector.tensor_sub(B4, mod_sb[:, 0:4], muA)

        # xm = xf*A + B (bf16); split DVE / Pool
        xm = []
        for i in range(NC_):
            xm_i = P.tile([128, T], bf16, tag=f"xm{i}")
            nc.vector.tensor_scalar(
                xm_i, xf[i], A4[:, i : i + 1], B4[:, i : i + 1], ALU.mult, ALU.add
            )
            xm.append(xm_i)
        fill(FILL_WARM)

        # ---- phase B: qkv matmul [1536, 1024] (channel order type-major) ----
        qkv = [P.tile([128, T], bf16, tag=f"qkv{m}", name=f"qkv{m}") for m in range(NO)]
        m_order = [p + 4 * ty for p in range(4) for ty in range(3)]

        def qkv_chunk(m):
            ps = SC.tile([128, T], f32, tag="sc", name=f"qkvps{m}")
            for t in range(NT):
                for j in range(4):
                    nc.tensor.matmul(
                        ps[:, 512 * t : 512 * (t + 1)],
                        qw[j][:, 128 * m : 128 * (m + 1)],
                        xm[j][:, 512 * t : 512 * (t + 1)],
                        start=(j == 0),
                        stop=(j == 3),
                    )
            # evictions all on DVE: ACT must stay exp-only once attention
            # starts (the scheduler may defer chunks into the attention
            # phase), and Pool can't read PSUM
            if m < 8:
                nc.vector.tensor_scalar_add(qkv[m][:], ps, qb_sb[:, m : m + 1])
            else:
                p = m - 8
                h0_, h1_ = 2 * p, 2 * p + 1
                # even head: rows 0..63 straight into its v80 source
                nc.vector.tensor_scalar_add(
                    v80[h0_][0:64, :], ps[0:64, :], qb_sb[0:64, m : m + 1]
                )
                # odd head: partition-aligned eviction, then DMA shift 64->0
                nc.vector.tensor_scalar_add(
                    vstage[p][64:128, :], ps[64:128, :], qb_sb[64:128, m : m + 1]
                )
                nc.sync.dma_start(out=v80[h1_][0:64, :], in_=vstage[p][64:128, :])
                for h in (h0_, h1_):
                    # whole-chunk XBAR transpose: vT[p, s, c] = v80[c, 128s+p]
                    nc.sync.dma_start_transpose(
                        out=vT[h][:, :, 0:80], in_=v80[h]
                    )

        # pair-0 chunks up front; the other 9 chunks are injected into
        # pair 0's attention loop (ACT lights up ~13us earlier this way)
        for m in (0, 4, 8):
            qkv_chunk(m)
        inject = [m for m in m_order if m not in (0, 4, 8)]
        for j in range(4):
            nc.sync.dma_start(out=pw[j], in_=pw_d.ap()[128 * j : 128 * (j + 1), :])

        # ---- phase C: attention, head pairs pipelined ----
        a_sb = [P.tile([128, T], bf16, tag=f"asb{j}", name=f"asb{j}") for j in range(NC_)]
        pend = []  # deferred recb/normalize from previous pair

        def emit_recb(items):
            # usb holds the bf16 eviction of U (rows 0..63 = PV, row 64 =
            # 1/denom); recb broadcasts 1/denom across partitions 0..63 via a
            # K=1 outer product.  The mul reads one PSUM operand only (walrus
            # constraint).
            for (h, usb_h) in items:
                recb = SC.tile([128, T], f32, tag="sc", name=f"recb{h}")
                for t in range(NT):
                    nc.tensor.matmul(
                        recb[0:64, 512 * t : 512 * (t + 1)],
                        ones_t[64:65, 0:64],
                        usb_h[64:65, 512 * t : 512 * (t + 1)],
                        start=True,
                        stop=True,
                        tile_position=(64, 0),
                    )
                off = 64 * (h % 2)
                if off == 0:
                    nc.vector.tensor_mul(
                        a_sb[h // 2][0:64, :], usb_h[0:64, :], recb[0:64, :]
                    )
                else:
                    a_t = ATMP.tile([64, T], bf16, tag="atmp", name=f"atmp{h}")
                    nc.vector.tensor_mul(a_t, usb_h[0:64, :], recb[0:64, :])
                    nc.sync.dma_start(out=a_sb[h // 2][64:128, :], in_=a_t)

        for p in range(4):
            h0, h1 = 2 * p, 2 * p + 1
            q_t, k_t, v_t = qkv[p], qkv[4 + p], qkv[8 + p]
            U = {}
            U[h0] = UU.tile([97, T], f32, tag="u", name=f"u{h0}")
            U[h1] = UU.tile([97, T], f32, tag="u", name=f"u{h1}")
            ex_tiles = {}
            sc_tiles = {}
            for s in range(NS):
                for h in (h0, h1):
                    off = 64 * (h % 2)
                    sc_ps = SC.tile([128, T], f32, tag="sc", name=f"sc{h}_{s}")
                    for t in range(NT):
                        nc.tensor.matmul(
                            sc_ps[:, 512 * t : 512 * (t + 1)],
                            k_t[off : off + 64, 128 * s : 128 * (s + 1)],
                            q_t[off : off + 64, 512 * t : 512 * (t + 1)],
                            start=True,
                            stop=True,
                            tile_position=(off, 0),
                        )
                    sc_tiles[(h, s)] = sc_ps
                # deferred normalize of the previous pair, once this pair's
                # first scores are in flight (recip latency already hidden).
                # Must happen before this pair's first U access (the s=0
                # filler) so the U buffer release sees all its accessors.
                if s == 0 and pend:
                    emit_recb(pend)
                    pend = []
                for h in (h0, h1):
                    ex = EXPP.tile([128, T], bf16, tag="ex")
                    nc.scalar.activation(ex, sc_tiles.pop((h, s)), AF.Exp, scale=0.125)
                    ex_tiles[(h, s)] = ex
                if s >= 1:
                    for h in (h0, h1):
                        ex = ex_tiles.pop((h, s - 1))
                        for t in range(NT):
                            nc.tensor.matmul(
                                U[h][0:65, 512 * t : 512 * (t + 1)],
                                vT[h][:, s - 1, 0:65],
                                ex[:, 512 * t : 512 * (t + 1)],
                                start=(s == 1),
                                stop=False,
                            )
                if inject:
                    qkv_chunk(inject.pop(0))
                    if s == 3 and inject:
                        qkv_chunk(inject.pop(0))
                else:
                    fill(FILL_C, U[h0])
            for h in (h0, h1):
                ex = ex_tiles.pop((h, NS - 1))
                for t in range(NT):
                    nc.tensor.matmul(
                        U[h][0:65, 512 * t : 512 * (t + 1)],
                        vT[h][:, NS - 1, 0:65],
                        ex[:, 512 * t : 512 * (t + 1)],
                        start=False,
                        stop=True,
                    )
            # evict U to SBUF bf16 (frees the PSUM banks early), then take
            # the denominator reciprocal in place on row 64
            new_pend = []
            for h in (h0, h1):
                usb_h = RECP.tile([65, T], bf16, tag="rec", name=f"usb{h}")
                nc.vector.tensor_copy(usb_h[0:64, :], U[h][0:64, :])
                nc.vector.reciprocal(usb_h[64:65, :], U[h][64:65, :])
                new_pend.append((h, usb_h))
            if p < 3:
                fill(FILL_BOUND, U[h0])
                pend = new_pend
            else:
                fill(FILL_TAIL, U[h0])
                emit_recb(new_pend)

        # ---- phase D: proj + gated residual ----
        # xfpb = xf + pb*gate (in place; raw xf only feeds the residual)
        pbg = P.tile([128, 4], f32, tag="pbg")
        nc.vector.tensor_mul(pbg, pb_sb, mod_sb[:, 8:12])
        for i in range(NC_):
            nc.vector.tensor_scalar_add(xf[i][:], xf[i][:], pbg[:, i : i + 1])
        for m in range(NC_):
            ps = SC.tile([128, T], f32, tag="sc", name=f"projps{m}")
            for t in range(NT):
                for j in range(4):
                    nc.tensor.matmul(
                        ps[:, 512 * t : 512 * (t + 1)],
                        pw[j][:, 128 * m : 128 * (m + 1)],
                        a_sb[j][:, 512 * t : 512 * (t + 1)],
                        start=(j == 0),
                        stop=(j == 3),
                    )
            out_m = OUTP.tile([128, T], f32, tag="outm", name=f"outm{m}")
            nc.vector.scalar_tensor_tensor(
                out_m, ps, mod_sb[:, 8 + m : 9 + m], xf[m][:], ALU.mult, ALU.add
            )
            nc.sync.dma_start(out=out_d.ap()[128 * m : 128 * (m + 1), :], in_=out_m)

    nc.compile()
    return nc


_PROGRAM = None
LAST_RESULTS = None


def _host_inputs(x, emb, qkv_w, qkv_b, ada_w, ada_b, proj_w, proj_b):
    import ml_dtypes

    x = np.asarray(x, np.float32)
    emb = np.asarray(emb, np.float32)
    perm = _perm()
    bf = ml_dtypes.bfloat16
    qkv_wT = np.ascontiguousarray(np.asarray(qkv_w, np.float32)[perm, :].T.astype(bf))
    qkv_b_p = np.ascontiguousarray(np.asarray(qkv_b, np.float32)[perm])
    ada_wT = np.ascontiguousarray(np.asarray(ada_w, np.float32).T.astype(bf))
    ada_b = np.ascontiguousarray(np.asarray(ada_b, np.float32))
    proj_wT = np.ascontiguousarray(np.asarray(proj_w, np.float32).T.astype(bf))
    proj_b = np.ascontiguousarray(np.asarray(proj_b, np.float32))

    gind = np.repeat(np.eye(8, dtype=np.float32), 16, axis=0) / 16.0  # [128, 8]
    gindT = np.ascontiguousarray(np.repeat(np.eye(8, dtype=np.float32), 16, axis=0).T)

    in_maps = []
    for b in range(B):
        smalls = np.zeros((128, 40), np.float32)
        smalls[:, 0:4] = emb[b].reshape(4, 128).T
        smalls[:, 4:16] = qkv_b_p.reshape(12, 128).T
        smalls[:, 16:28] = ada_b.reshape(12, 128).T
        smalls[:, 28:32] = proj_b.reshape(4, 128).T
        smalls[:, 32:40] = gind
        in_maps.append(
            {
                "x": np.ascontiguousarray(x[b].reshape(C, T)),
                "smalls": smalls,
                "qkv_wT": qkv_wT,
                "ada_wT": ada_wT,
                "proj_wT": proj_wT,
                "gindT": gindT,
            }
        )
    return in_maps


def _get_program():
    global _PROGRAM
    if _PROGRAM is None:
        _PROGRAM = _build_program()
    return _PROGRAM


def kernel(x, emb, qkv_w, qkv_b, ada_w, ada_b, proj_w, proj_b, _trace=False):
    global LAST_RESULTS
    nc = _get_program()
    in_maps = _host_inputs(x, emb, qkv_w, qkv_b, ada_w, ada_b, proj_w, proj_b)
    res = run_bass_kernel_spmd(nc, in_maps, list(range(8)), trace=_trace)
    LAST_RESULTS = res
    out = np.stack([res.results[b]["out"] for b in range(B)], axis=0)
    return np.ascontiguousarray(out.reshape(B, C, HH, WW).astype(np.float32))
